# revision 4
# baseline (speedup 1.0000x reference)
"""Trainium2 Bass kernel for: conv3x3(64->64) -> conv3x3(64->64) -> maxpool2x2
-> per-tile(4x4) global max pool -> sum over tiles.  x: [4,64,512,512] f32.

Sharding: data-parallel over 8 NeuronCores; core c handles rows
[256*(c%2), 256*(c%2)+256) of image c//2 (256 conv2-output rows each), with a
4-row halo pre-baked into its input shard by the host.

On-core strategy: convs run on the TensorEngine in bf16 (fp32 PSUM
accumulation; final error ~1e-3 relative).  x is stored in a "parity" layout:
partitions 0-63 = channels of even rows, 64-127 = odd rows, free dim =
(row-pair slot) x (padded width 514).  Per output row-pair and kx tap:
  - one full [128,128]-weight matmul computes all four (input parity x output
    parity) ky-tap combinations at once;
  - two K=64 "leftover" matmuls add the halo rows above/below the pair,
    placed in distinct 64x64 PE-array quadrants so they run concurrently.
conv1's PSUM output-parity placement alternates per pair ("swap") so leftover
matmuls of adjacent pairs cover all 4 quadrants; the swapped layout is left
as-is in SBUF (single ScalarE copy per pair) and conv2's weight blocks are
built to compensate for each slot's parity placement.
maxpool2x2 + per-tile global max collapse into one running elementwise max
over conv2 psum tiles on the VectorEngine (maxpool then tile-max == max over
the whole 128x128-pooled region).
"""
import numpy as np
import ml_dtypes

import concourse.bacc as bacc
import concourse.mybir as mybir
import concourse.tile as tile
from concourse.bass_utils import run_bass_kernel_spmd

BF16 = mybir.dt.bfloat16
F32 = mybir.dt.float32
NPBF16 = ml_dtypes.bfloat16

N_CORES = 8
C = 64
H = 512
W = 512
WP = W + 2                  # padded row width in SBUF
S = 64                      # conv2 output rows per strip
NSTRIP = (H // 2) // S      # strips per core
XS_SLOTS = (S + 8) // 2     # 36 row-pair slots per x strip tile
C1_SLOTS = (S + 4) // 2     # 34 row-pair slots per conv1 strip tile
XR = 256 + 8                # rows in the per-core x shard (4 halo rows each side)
XSLOTS_TOT = XR // 2        # 132 row-pair slots in the shard

# weight tile column layout (bf16, [128, NWCOL]):
#   full blocks [128,128]: conv1 (si=0, so in {0,1}), conv2 (si in {0,1}, so=0)
#     col = FULL_IDX[(l, si, so)] * 128
#   ab blocks [128,64]: AB_IDX[(l, variant)]; variant 0 = (ky2 @ parts 0-63,
#     ky0 @ 64-127), variant 1 = mirrored.  conv1 uses variant 0 only.
FULL_KEYS = [(0, 0, 0), (0, 0, 1), (1, 0, 0), (1, 1, 0)]
AB_KEYS = [(0, 0), (1, 0), (1, 1)]
NWCOL = len(FULL_KEYS) * 3 * 128 + len(AB_KEYS) * 3 * 64   # 2112


def _full_col(l, si, so, kx):
    return (FULL_KEYS.index((l, si, so)) * 3 + kx) * 128


def _ab_col(l, var, kx):
    return len(FULL_KEYS) * 3 * 128 + (AB_KEYS.index((l, var)) * 3 + kx) * 64


def build_weights(w1, w2):
    """Pack w1, w2 [64,64,3,3] (OIHW fp32) into the [128, NWCOL] bf16 tile."""
    ws = {0: w1, 1: w2}
    wt = np.zeros((128, NWCOL), dtype=np.float32)
    for (l, si, so) in FULL_KEYS:
        w = ws[l]
        for kx in range(3):
            blk = np.zeros((128, 128), dtype=np.float32)
            for ph in (0, 1):
                for qh in (0, 1):
                    ky = (ph ^ si) - (qh ^ so) + 1
                    assert 0 <= ky <= 2
                    blk[ph * 64:(ph + 1) * 64,
                        qh * 64:(qh + 1) * 64] = w[:, :, ky, kx].T
            c0 = _full_col(l, si, so, kx)
            wt[:, c0:c0 + 128] = blk
    for (l, var) in AB_KEYS:
        w = ws[l]
        for kx in range(3):
            ab = np.zeros((128, 64), dtype=np.float32)
            ky_lo, ky_hi = (2, 0) if var == 0 else (0, 2)
            ab[0:64] = w[:, :, ky_lo, kx].T
            ab[64:128] = w[:, :, ky_hi, kx].T
            c0 = _ab_col(l, var, kx)
            wt[:, c0:c0 + 64] = ab
    return wt.astype(NPBF16)


def shard_input(x):
    """x [4,64,512,512] f32 -> 8 bf16 shards [128, XSLOTS_TOT*WP] in parity
    layout (partition = 64*row_parity + channel, free = slot*WP + col+1)."""
    xb = x.astype(NPBF16)
    shards = []
    for c in range(N_CORES):
        n, h = divmod(c, 2)
        sh = np.zeros((C, XR, WP), dtype=NPBF16)
        r0 = h * 256 - 4
        lo, hi = max(r0, 0), min(r0 + XR, H)
        sh[:, lo - r0:hi - r0, 1:1 + W] = xb[n, :, lo:hi, :]
        flat = np.concatenate(
            [sh[:, 0::2, :].reshape(C, -1), sh[:, 1::2, :].reshape(C, -1)], axis=0)
        shards.append(np.ascontiguousarray(flat))
    return shards


def _emit_superblock(nc, xt, wt, pool, tag, l, pair0, slot_of_pair, in_swap,
                     out_swap):
    """Emit the 18 matmuls for 2 adjacent row-pairs of conv layer l.

    in_swap(slot) -> 0/1: which parity placement the input slot uses
      (0: even rows at partitions 0-63).
    out_swap(pair) -> 0/1: psum placement for this pair's output
      (0: even output row at psum partitions 0-63).
    Returns [(pair, psum_tile), ...].
    """
    tiles = [(pr, pool.tile([128, 512], F32, tag=tag, name=tag))
             for pr in (pair0, pair0 + 1)]
    for pr, pt in tiles:
        j = slot_of_pair(pr)
        si, so = in_swap(j), out_swap(pr)
        for kx in range(3):
            fc = _full_col(l, si, so, kx)
            nc.tensor.matmul(
                pt[:], wt[:, fc:fc + 128],
                xt[:, j * WP + kx: j * WP + kx + 512],
                start=(kx == 0), stop=False, skip_group_check=True)
    for kx in range(3):
        for pr, pt in tiles:
            j = slot_of_pair(pr)
            so = out_swap(pr)
            # "a": out-even += ky0 * (odd in-row of slot j-1)
            sa = in_swap(j - 1)
            a_base = (1 - sa) * 64            # odd rows live at parts (1-sa)*64
            a_out = so * 64
            a_col = _ab_col(l, 1 if a_base == 0 else 0, kx)
            nc.tensor.matmul(
                pt[a_out:a_out + 64],
                wt[a_base:a_base + 64, a_col:a_col + 64],
                xt[a_base:a_base + 64,
                   (j - 1) * WP + kx: (j - 1) * WP + kx + 512],
                start=False, stop=False, skip_group_check=True)
            # "b": out-odd += ky2 * (even in-row of slot j+1)
            sb_ = in_swap(j + 1)
            b_base = sb_ * 64                 # even rows live at parts sb*64
            b_out = (1 - so) * 64
            b_col = _ab_col(l, 1 if b_base == 64 else 0, kx)
            nc.tensor.matmul(
                pt[b_out:b_out + 64],
                wt[b_base:b_base + 64, b_col:b_col + 64],
                xt[b_base:b_base + 64,
                   (j + 1) * WP + kx: (j + 1) * WP + kx + 512],
                start=False, stop=(kx == 2), skip_group_check=True)
    return tiles


def build_kernel(repeat=1):
    nc = bacc.Bacc("TRN2", target_bir_lowering=False, debug=False,
                   num_devices=N_CORES)
    xs_d = nc.dram_tensor("xs", [128, XSLOTS_TOT * WP], BF16,
                          kind="ExternalInput").ap()
    wt_d = nc.dram_tensor("wt", [128, NWCOL], BF16, kind="ExternalInput").ap()
    out_d = nc.dram_tensor("out", [C, 1], F32, kind="ExternalOutput").ap()

    # persistent sbuf tensors (allocated outside the Tile pools)
    wt = nc.alloc_sbuf_tensor("wt_sb", [128, NWCOL], BF16).ap()
    c1bufs = [nc.alloc_sbuf_tensor(f"c1_{i}", [128, C1_SLOTS * WP], BF16).ap()
              for i in range(2)]
    mxacc = nc.alloc_sbuf_tensor("mxacc", [128, 512], F32).ap()
    results = nc.alloc_sbuf_tensor("results", [128, 8], F32).ap()
    res_hi = nc.alloc_sbuf_tensor("res_hi", [64, 8], F32).ap()
    res64 = nc.alloc_sbuf_tensor("res64", [64, 8], F32).ap()
    partial = nc.alloc_sbuf_tensor("partial", [64, 1], F32).ap()

    with tile.TileContext(nc) as tc:
        import contextlib
        with contextlib.ExitStack() as ctx:
            xpool = ctx.enter_context(tc.tile_pool(name="xsp", bufs=2))
            p1pool = ctx.enter_context(tc.tile_pool(name="p1", bufs=4, space="PSUM"))
            p2pool = ctx.enter_context(tc.tile_pool(name="p2", bufs=4, space="PSUM"))

            nc.sync.dma_start(wt[:], wt_d[:])
            # zero pad columns (0 and WP-1 of each slot) of conv1 strip bufs;
            # conv1 copies only ever write cols 1..512 of a slot.
            for buf in c1bufs:
                b3 = buf.rearrange("p (k w) -> p k w", w=WP)
                nc.vector.memset(b3[:, :, 0:1], 0.0)
                nc.vector.memset(b3[:, :, WP - 1:WP], 0.0)

            def body():
                for s in range(NSTRIP):
                    c1 = c1bufs[s % 2]
                    xt = xpool.tile([128, XS_SLOTS * WP], BF16, tag="xs_t",
                                    name="xs_t")
                    nc.sync.dma_start(
                        xt[:],
                        xs_d[:, (s * S // 2) * WP: (s * S // 2 + XS_SLOTS) * WP])

                    # conv1: pairs 0..C1_SLOTS-1 (rows 64s-2 .. 64s+65);
                    # x slot of pair m is m+1; x slots are never swapped;
                    # conv1 psum (and hence c1 slot m) swap = m % 2.
                    for m0 in range(0, C1_SLOTS, 2):
                        tiles = _emit_superblock(
                            nc, xt, wt, p1pool, "p1t", 0, m0,
                            lambda pr: pr + 1,
                            in_swap=lambda j: 0,
                            out_swap=lambda pr: pr % 2)
                        for pr, pt in tiles:
                            nc.scalar.copy(
                                c1[:, pr * WP + 1: pr * WP + 513], pt[:])

                    # conv2 + running max: pairs 0..S/2-1; c1 slot of pair b
                    # is b+1 (slot swap = slot % 2); conv2 psum never swapped.
                    for b0 in range(0, S // 2, 2):
                        tiles = _emit_superblock(
                            nc, c1, wt, p2pool, "p2t", 1, b0,
                            lambda pr: pr + 1,
                            in_swap=lambda j: j % 2,
                            out_swap=lambda pr: 0)
                        for pr, pt in tiles:
                            gb = s * (S // 2) + pr
                            if gb % 64 == 0:
                                nc.vector.tensor_copy(mxacc[:], pt[:])
                            else:
                                nc.vector.tensor_max(mxacc[:], mxacc[:], pt[:])
                            if gb % 64 == 63:
                                t = gb // 64
                                for seg in range(4):
                                    nc.vector.reduce_max(
                                        results[:, t * 4 + seg: t * 4 + seg + 1],
                                        mxacc[:, seg * 128:(seg + 1) * 128],
                                        axis=mybir.AxisListType.X)

                # combine row-parity halves: partitions 64-127 -> 0-63 via a
                # small SBUF->SBUF DMA (engines can't mix base partitions).
                nc.sync.dma_start(res_hi[:], results[64:128, :])
                nc.vector.tensor_max(res64[:], results[0:64, :], res_hi[:])
                nc.vector.reduce_sum(partial[:], res64[:],
                                     axis=mybir.AxisListType.X)
                nc.sync.dma_start(out_d[:], partial[:])

            if repeat == 1:
                body()
            else:
                with tc.For_i(0, repeat, 1):
                    body()
    nc.compile()
    return nc


_CACHE = {}


def _get_nc(repeat=1):
    if repeat not in _CACHE:
        _CACHE[repeat] = build_kernel(repeat)
    return _CACHE[repeat]


def run_cores(x, w1, w2, repeat=1):
    nc = _get_nc(repeat)
    wt = build_weights(np.asarray(w1, dtype=np.float32),
                       np.asarray(w2, dtype=np.float32))
    shards = shard_input(np.asarray(x, dtype=np.float32))
    in_maps = [{"xs": shards[c], "wt": wt} for c in range(N_CORES)]
    return run_bass_kernel_spmd(nc, in_maps, core_ids=list(range(N_CORES)))


def kernel(x, w1, w2, H=None, W=None, nTh=None, nTw=None, **_):
    res = run_cores(x, w1, w2)
    out = np.zeros((4, C, 1, 1), dtype=np.float32)
    for n in range(4):
        out[n, :, 0, 0] = (res.results[2 * n]["out"][:, 0]
                           + res.results[2 * n + 1]["out"][:, 0])
    return out
